# revision 1
# baseline (speedup 1.0000x reference)
"""Trainium2 Bass kernel for nn_BasicNet4 (Emformer encoder, sparse attention).

Strategy:
  - Data-parallel over batch B=8 across 8 NeuronCores (weights replicated).
  - Tokens reordered host-side into segment-interleaved order:
    seg i -> [rc_i, u_{4i}, u_{4i+1}, u_{4i+2}, u_{4i+3}]  (5 tokens x 256 segs = 1280)
    so attention is block-diagonal with 5x5 blocks.
  - Activations kept transposed in SBUF: [d on partitions (4 tiles of 128), tokens on free].
  - LayerNorm gains/biases folded into weights host-side; LN stats computed with
    ones-matmul partition reductions on the PE (broadcast form).
  - Attention masks folded into the score matmuls as extra low-rank (+/-C indicator)
    contraction terms; block-diagonal attention computed per 128-token diagonal tile
    plus small "halo" edge strips.
  - bf16 matmul operands / residual stream, fp32 PSUM accumulation.
"""

import sys

sys.path.insert(0, "/opt/trn_rl_repo")

import numpy as np
import ml_dtypes

import concourse.bass as bass
import concourse.mybir as mybir
import concourse.tile as tile
from concourse import bass_utils, bacc

bf16 = ml_dtypes.bfloat16
dt = mybir.dt
AF = mybir.ActivationFunctionType
ALU = mybir.AluOpType

# Model config (hardcoded from the problem spec)
D, H, FFN, L = 512, 4, 128, 4
SEG, RC = 4, 1
B, T = 8, 1025
U = T - RC            # 1024
NSEG = U // SEG       # 256
TT = NSEG * (SEG + RC)  # 1280 interleaved tokens
NT = TT // 128        # 10 token tiles
DT = D // 128         # 4 d tiles
DH = D // H           # 128 (= one partition tile per head)
NCORES = 8
CHUNKS = [(0, 512), (512, 512), (1024, 256)]  # free-dim chunks <= 512 (one PSUM bank)

CBF = np.float32(bf16(np.float32(1e9)))  # mask constant, exactly representable in bf16

_COMPILED = None


def _tok_index():
    # interleaved token t -> original frame index in x[:, :T]
    t = np.arange(TT)
    seg = t // 5
    pos = t % 5
    off = np.array([4, 0, 1, 2, 3])[pos]
    return 4 * seg + off  # in [0, 1024]


def _qt_geometry(qt):
    """MID window is the aligned [128qt, 128qt+128). LEFT/RIGHT edges are the
    few extra k-tokens of the straddling segments."""
    q0 = 128 * qt
    sk = 5 * (q0 // 5)
    op = q0 - sk                      # 0..4
    ek = min(5 * (-(-(q0 + 128) // 5)), TT)
    nL = op                           # left edge width (tokens [sk, q0))
    nR = max(ek - (q0 + 128), 0)      # right edge width (tokens [q0+128, ek))
    return q0, sk, op, nL, nR


def _mask_consts():
    """Per-qt mask matmul operands (host-computed, bf16).
    MID:  Lmid[qt] [128,128] (lhsT), Rmid[qt] [128,128] (rhs):
          sum_r Lmid[r,k]*Rmid[r,j] = -C + C*[seg(k)==seg(j)]  (window-local segs)
    EDGE: Lel[qt] [128,8], Rel[qt] [128,128]: same for the 8 edge slots
          (slots 0..3 = LEFT tokens, 4..7 = RIGHT tokens; invalid slots -> -C only).
    """
    Lmid = np.zeros((NT, 128, 128), np.float32)
    Rmid = np.zeros((NT, 128, 128), np.float32)
    Lel = np.zeros((NT, 128, 8), np.float32)
    Rel = np.zeros((NT, 128, 128), np.float32)
    for qt in range(NT):
        q0, sk, op, nL, nR = _qt_geometry(qt)
        segq = (op + np.arange(128)) // 5       # window-local seg of q (and mid k)
        # MID
        Lmid[qt, 0, :] = 1.0
        Rmid[qt, 0, :] = -CBF
        nseg = segq[-1] + 1
        for i in range(nseg):
            Lmid[qt, 1 + i, :] = (segq == i)
            Rmid[qt, 1 + i, :] = CBF * (segq == i)
        # EDGE
        Lel[qt, 0, :] = 1.0
        Rel[qt, 0, :] = -CBF
        slot_seg = np.full(8, -1)
        for s in range(nL):
            slot_seg[s] = 0                      # left tokens are in window-local seg 0
        for s in range(nR):
            slot_seg[4 + s] = (128 + op + s) // 5
        esegs = sorted(set(slot_seg[slot_seg >= 0]))
        for j, e in enumerate(esegs):
            Lel[qt, 1 + j, :] = (slot_seg == e)
            Rel[qt, 1 + j, :] = CBF * (segq == e)
    return Lmid.astype(bf16), Rmid.astype(bf16), Lel.astype(bf16), Rel.astype(bf16)


def _host_prep(ins):
    """Fold LN affines into weights, transpose, cast. Returns shared input map."""
    f32 = np.float32
    m = {}
    scale = np.float32(DH) ** -0.5
    for l in range(L):
        g_i, b_i = f32(ins["ln_in_g"][l]), f32(ins["ln_in_b"][l])
        g_f, b_f = f32(ins["ff_ln_g"][l]), f32(ins["ff_ln_b"][l])
        Wq = f32(ins["Wq"][l]);  bq = f32(ins["bq"][l])
        Wk = f32(ins["Wkv"][l][:D]);  bk = f32(ins["bkv"][l][:D])
        Wv = f32(ins["Wkv"][l][D:]);  bv = f32(ins["bkv"][l][D:])
        Wo = f32(ins["Wo"][l]);  bo = f32(ins["bo"][l])
        W1 = f32(ins["W1"][l]);  b1 = f32(ins["b1"][l])
        W2 = f32(ins["W2"][l]);  b2 = f32(ins["b2"][l])
        Wq_ = scale * (Wq * g_i[None, :]); bq_ = scale * (bq + Wq @ b_i)
        Wk_ = Wk * g_i[None, :];           bk_ = bk + Wk @ b_i
        Wv_ = Wv * g_i[None, :];           bv_ = bv + Wv @ b_i
        W1_ = W1 * g_f[None, :];           b1_ = b1 + W1 @ b_f
        m[f"wq{l}"] = Wq_.T.copy().astype(bf16)   # [din, dout]
        m[f"wk{l}"] = Wk_.T.copy().astype(bf16)
        m[f"wv{l}"] = Wv_.T.copy().astype(bf16)
        m[f"wo{l}"] = Wo.T.copy().astype(bf16)
        m[f"w1{l}"] = W1_.T.copy().astype(bf16)   # [512, 128]
        m[f"w2{l}"] = W2.T.copy().astype(bf16)    # [128, 512]
        m[f"bq{l}"] = bq_.reshape(DT, 128).T.copy()       # [128, DT] f32 per-partition
        m[f"bk{l}"] = bk_.reshape(DT, 128).T.copy()
        m[f"bv{l}"] = bv_.reshape(1, D).astype(bf16)      # [1, 512] row (K=1 matmul)
        m[f"bo{l}"] = bo.reshape(DT, 128).T.copy()
        m[f"b1{l}"] = b1_.reshape(1, 128).T.copy()        # [128, 1]
        m[f"b2{l}"] = b2.reshape(DT, 128).T.copy()
        m[f"go{l}"] = f32(ins["ln_out_g"][l]).reshape(DT, 128).T.copy()
        m[f"bo2{l}"] = f32(ins["ln_out_b"][l]).reshape(DT, 128).T.copy()
    Lmid, Rmid, Lel, Rel = _mask_consts()
    m["lmid"] = np.ascontiguousarray(Lmid.transpose(1, 0, 2))  # [128, NT, 128]
    m["rmid"] = np.ascontiguousarray(Rmid.transpose(1, 0, 2))
    m["lel"] = np.ascontiguousarray(Lel.transpose(1, 0, 2))    # [128, NT, 8]
    m["rel"] = np.ascontiguousarray(Rel.transpose(1, 0, 2))
    m["ones_c"] = np.full((128, 128), 1.0 / D, bf16)           # stats lhsT (bcast reduce)
    m["allones"] = np.ones((128, 128), bf16)                   # denominator lhsT
    m["ones1"] = np.ones((1, 128), bf16)                       # K=1 bcast lhsT
    m["ident"] = np.eye(128, dtype=bf16)                       # residual adds
    return m


def _dram_inputs(nc):
    a = {}
    def inp(name, shape, dtype):
        a[name] = nc.dram_tensor(name, list(shape), dtype, kind="ExternalInput").ap()
    inp("xT", (D, TT), dt.bfloat16)
    for l in range(L):
        inp(f"wq{l}", (D, D), dt.bfloat16); inp(f"wk{l}", (D, D), dt.bfloat16)
        inp(f"wv{l}", (D, D), dt.bfloat16); inp(f"wo{l}", (D, D), dt.bfloat16)
        inp(f"w1{l}", (D, FFN), dt.bfloat16); inp(f"w2{l}", (FFN, D), dt.bfloat16)
        inp(f"bq{l}", (128, DT), dt.float32); inp(f"bk{l}", (128, DT), dt.float32)
        inp(f"bv{l}", (1, D), dt.bfloat16); inp(f"bo{l}", (128, DT), dt.float32)
        inp(f"b1{l}", (128, 1), dt.float32); inp(f"b2{l}", (128, DT), dt.float32)
        inp(f"go{l}", (128, DT), dt.float32); inp(f"bo2{l}", (128, DT), dt.float32)
    inp("lmid", (128, NT, 128), dt.bfloat16); inp("rmid", (128, NT, 128), dt.bfloat16)
    inp("lel", (128, NT, 8), dt.bfloat16); inp("rel", (128, NT, 128), dt.bfloat16)
    inp("ones_c", (128, 128), dt.bfloat16); inp("allones", (128, 128), dt.bfloat16)
    inp("ones1", (1, 128), dt.bfloat16); inp("ident", (128, 128), dt.bfloat16)
    out = nc.dram_tensor("out", [128, DT], dt.float32, kind="ExternalOutput").ap()
    return a, out


def _ln_normalize(nc, acts, sbufs, psums, smalls, cat, z_out, eps_tile):
    """z = (cat - mean) * rstd in bcast form. cat/z: [128, DT, TT] bf16 sbuf."""
    ones_c = smalls["ones_c"]
    # squares on GPSIMD (bf16)
    sq = acts.tile([128, DT, TT], dt.bfloat16, tag="sq")
    for d in range(DT):
        nc.gpsimd.tensor_tensor(sq[:, d], cat[:, d], cat[:, d], ALU.mult)
    p_mu = psums.tile([128, TT], dt.float32, tag="big")
    p_e2 = psums.tile([128, TT], dt.float32, tag="big")
    for (c0, cn) in CHUNKS:
        for d in range(DT):
            nc.tensor.matmul(p_mu[:, c0:c0 + cn], ones_c[:], cat[:, d, c0:c0 + cn],
                             start=(d == 0), stop=(d == DT - 1))
        for d in range(DT):
            nc.tensor.matmul(p_e2[:, c0:c0 + cn], ones_c[:], sq[:, d, c0:c0 + cn],
                             start=(d == 0), stop=(d == DT - 1))
    # Note ones_c = 1/512 -> p_mu = mean, p_e2 = E[x^2]; all rows identical (allones trick
    # not needed: ones_c is [128,1] so out partition count is... see below)
    mu_b = sbufs.tile([128, TT], dt.bfloat16, tag="mu")
    sq_mu = sbufs.tile([128, TT], dt.float32, tag="sqmu")
    var = sbufs.tile([128, TT], dt.float32, tag="var")
    A = sbufs.tile([128, TT], dt.bfloat16, tag="A")
    nc.scalar.activation(sq_mu[:], p_mu[:], AF.Square)
    nc.vector.tensor_copy(mu_b[:], p_mu[:])
    nc.vector.tensor_tensor(var[:], p_e2[:], sq_mu[:], ALU.subtract)
    nc.scalar.activation(var[:], var[:], AF.Sqrt, bias=eps_tile[:], scale=1.0)
    with nc.allow_low_precision(reason="per-token rstd in bf16 is fine here"):
        nc.vector.reciprocal(A[:], var[:])
    for d in range(DT):
        z1 = sbufs.tile([128, TT], dt.bfloat16, tag="z1")
        nc.vector.tensor_tensor(z1[:], cat[:, d], mu_b[:], ALU.subtract)
        nc.vector.tensor_tensor(z_out[:, d], z1[:], A[:], ALU.mult)


def _trace(nc):
    a, out_dram = _dram_inputs(nc)
    with tile.TileContext(nc) as tc:
        import contextlib
        ctx = contextlib.ExitStack()
        with ctx:
            consts = ctx.enter_context(tc.tile_pool(name="consts", bufs=1))
            wpool = ctx.enter_context(tc.tile_pool(name="w", bufs=2))
            acts = ctx.enter_context(tc.tile_pool(name="acts", bufs=1))
            sbufs = ctx.enter_context(tc.tile_pool(name="sbufs", bufs=2))
            psums = ctx.enter_context(tc.tile_pool(name="psums", bufs=2, space="PSUM"))
            pv = ctx.enter_context(tc.tile_pool(name="pv", bufs=2, space="PSUM"))

            # constants
            smalls = {}
            for name, shape, dd in [
                ("lmid", [128, NT, 128], dt.bfloat16), ("rmid", [128, NT, 128], dt.bfloat16),
                ("lel", [128, NT, 8], dt.bfloat16), ("rel", [128, NT, 128], dt.bfloat16),
                ("ones_c", [128, 128], dt.bfloat16), ("allones", [128, 128], dt.bfloat16),
                ("ones1", [1, 128], dt.bfloat16), ("ident", [128, 128], dt.bfloat16),
            ]:
                t = consts.tile(shape, dd, tag=name)
                nc.sync.dma_start(t[:], a[name])
                smalls[name] = t
            eps_tile = consts.tile([128, 1], dt.float32)
            nc.vector.memset(eps_tile[:], 1e-5)

            # initial residual stream (transposed, interleaved)
            cat = acts.tile([128, DT, TT], dt.bfloat16, tag="cat0")
            nc.sync.dma_start(cat[:], a["xT"].rearrange("(dtile p) t -> p dtile t", p=128))

            for l in range(L):
                # --- load layer weights ---
                w = {}
                for nm, shape in [("wq", [128, DT, D]), ("wk", [128, DT, D]),
                                  ("wv", [128, DT, D]), ("wo", [128, DT, D]),
                                  ("w1", [128, DT, FFN]), ("w2", [128, D])]:
                    t = wpool.tile(shape, dt.bfloat16, tag=nm)
                    src = a[f"{nm}{l}"]
                    if nm == "w2":
                        nc.sync.dma_start(t[:], src)
                    else:
                        nc.sync.dma_start(t[:], src.rearrange("(dtile p) o -> p dtile o", p=128))
                    w[nm] = t
                bias = {}
                for nm in ["bq", "bk", "bo", "b1", "b2", "go", "bo2"]:
                    t = wpool.tile([128, DT] if nm != "b1" else [128, 1], dt.float32, tag=nm)
                    nc.sync.dma_start(t[:], a[f"{nm}{l}"])
                    bias[nm] = t
                bv = wpool.tile([1, D], dt.bfloat16, tag="bv")
                nc.sync.dma_start(bv[:], a[f"bv{l}"])

                # --- ln_in -> z ---
                z = acts.tile([128, DT, TT], dt.bfloat16, tag="z")
                _ln_normalize(nc, acts, sbufs, psums, smalls, cat, z, eps_tile)

                # --- Q, K projections (weights stationary -> transposed out) ---
                qk = {}
                for nm, bnm in [("wq", "bq"), ("wk", "bk")]:
                    dst = acts.tile([128, DT, TT], dt.bfloat16, tag="q" if nm == "wq" else "k")
                    for o in range(DT):
                        p = psums.tile([128, TT], dt.float32, tag="big")
                        for (c0, cn) in CHUNKS:
                            for d in range(DT):
                                nc.tensor.matmul(
                                    p[:, c0:c0 + cn],
                                    w[nm][:, d, 128 * o:128 * o + 128],
                                    z[:, d, c0:c0 + cn],
                                    start=(d == 0), stop=(d == DT - 1))
                        nc.scalar.activation(dst[:, o], p[:], AF.Identity,
                                             bias=bias[bnm][:, o:o + 1], scale=1.0)
                    qk[nm] = dst
                q_t, k_t = qk["wq"], qk["wk"]

                # --- V projection (acts stationary -> natural out [t, d]) ---
                v_nat = acts.tile([128, NT, D], dt.bfloat16, tag="v")
                for tt_i in range(NT):
                    p = pv.tile([128, D], dt.float32, tag="small")
                    for d in range(DT):
                        nc.tensor.matmul(p[:], z[:, d, 128 * tt_i:128 * tt_i + 128],
                                         w["wv"][:, d, :], start=(d == 0), stop=False)
                    nc.tensor.matmul(p[:], smalls["ones1"][:, 128 * 0:128], bv[:],
                                     start=False, stop=True)
                    nc.vector.tensor_copy(v_nat[:, tt_i], p[:])

                # --- halos (K columns easy; V rows via partition-offset copies) ---
                k_halo = sbufs.tile([128, DT, NT, 8], dt.bfloat16, tag="khalo")
                v_halo = acts.tile([8, NT, D], dt.bfloat16, tag="vhalo")
                nc.gpsimd.memset(k_halo[:], 0.0)
                nc.gpsimd.memset(v_halo[:], 0.0)
                for qt in range(NT):
                    q0, sk, op, nL, nR = _qt_geometry(qt)
                    if nL > 0:
                        nc.gpsimd.tensor_copy(k_halo[:, :, qt, 0:nL], k_t[:, :, sk:sk + nL])
                        nc.sync.dma_start(v_halo[0:nL, qt, :],
                                          v_nat[128 - nL:128, qt - 1, :])
                    if nR > 0:
                        nc.gpsimd.tensor_copy(k_halo[:, :, qt, 4:4 + nR],
                                              k_t[:, :, q0 + 128:q0 + 128 + nR])
                        nc.sync.dma_start(v_halo[4:4 + nR, qt, :],
                                          v_nat[0:nR, qt + 1, :])

                # --- attention per head ---
                attn = acts.tile([128, DT, TT], dt.bfloat16, tag="attn")
                for h in range(H):
                    p_mid = psums.tile([128, TT], dt.float32, tag="big")
                    p_edge = psums.tile([128, TT], dt.float32, tag="big")
                    for qt in range(NT):
                        q0 = 128 * qt
                        qs = q_t[:, h, q0:q0 + 128]
                        nc.tensor.matmul(p_mid[:, q0:q0 + 128], smalls["lmid"][:, qt],
                                         smalls["rmid"][:, qt], start=True, stop=False)
                        nc.tensor.matmul(p_mid[:, q0:q0 + 128], k_t[:, h, q0:q0 + 128],
                                         qs, start=False, stop=True)
                        nc.tensor.matmul(p_edge[0:8, q0:q0 + 128], smalls["lel"][:, qt],
                                         smalls["rel"][:, qt], start=True, stop=False)
                        nc.tensor.matmul(p_edge[0:8, q0:q0 + 128], k_halo[:, h, qt], qs,
                                         start=False, stop=True)
                    pa = sbufs.tile([128, TT], dt.bfloat16, tag="pa")
                    pe = sbufs.tile([8, TT], dt.bfloat16, tag="pe")
                    nc.scalar.activation(pa[:], p_mid[:], AF.Exp)
                    nc.scalar.activation(pe[:], p_edge[0:8, :], AF.Exp)
                    # denominator (broadcast over partitions via all-ones lhsT)
                    p_den = psums.tile([128, TT], dt.float32, tag="big")
                    for (c0, cn) in CHUNKS:
                        nc.tensor.matmul(p_den[:, c0:c0 + cn], smalls["allones"][:],
                                         pa[:, c0:c0 + cn], start=True, stop=False)
                        nc.tensor.matmul(p_den[:, c0:c0 + cn], smalls["allones"][0:8],
                                         pe[:, c0:c0 + cn], start=False, stop=True)
                    rec = sbufs.tile([128, TT], dt.bfloat16, tag="rec")
                    with nc.allow_low_precision(reason="softmax denom recip in bf16"):
                        nc.vector.reciprocal(rec[:], p_den[:])
                    # attn value matmuls
                    p_av = psums.tile([128, TT], dt.float32, tag="big")
                    for qt in range(NT):
                        q0 = 128 * qt
                        nc.tensor.matmul(p_av[:, q0:q0 + 128], v_nat[:, qt, 128 * h:128 * h + 128],
                                         pa[:, q0:q0 + 128], start=True, stop=False)
                        nc.tensor.matmul(p_av[:, q0:q0 + 128], v_halo[:, qt, 128 * h:128 * h + 128],
                                         pe[:, q0:q0 + 128], start=False, stop=True)
                    nc.vector.tensor_tensor(attn[:, h], p_av[:], rec[:], ALU.mult)

                # --- Wo projection + residual ---
                rc = acts.tile([128, DT, TT], dt.bfloat16, tag=f"cat{(l + 1) % 2}")
                for o in range(DT):
                    p = psums.tile([128, TT], dt.float32, tag="big")
                    for (c0, cn) in CHUNKS:
                        for d in range(DT):
                            nc.tensor.matmul(p[:, c0:c0 + cn],
                                             w["wo"][:, d, 128 * o:128 * o + 128],
                                             attn[:, d, c0:c0 + cn],
                                             start=(d == 0), stop=False)
                        nc.tensor.matmul(p[:, c0:c0 + cn], smalls["ident"][:],
                                         cat[:, o, c0:c0 + cn], start=False, stop=True)
                    nc.scalar.activation(rc[:, o], p[:], AF.Identity,
                                         bias=bias["bo"][:, o:o + 1], scale=1.0)

                # --- ff_ln -> zf ---
                zf = acts.tile([128, DT, TT], dt.bfloat16, tag="z")
                _ln_normalize(nc, acts, sbufs, psums, smalls, rc, zf, eps_tile)

                # --- FFN ---
                h1 = acts.tile([128, TT], dt.bfloat16, tag="h1")
                p = psums.tile([128, TT], dt.float32, tag="big")
                for (c0, cn) in CHUNKS:
                    for d in range(DT):
                        nc.tensor.matmul(p[:, c0:c0 + cn], w["w1"][:, d, :],
                                         zf[:, d, c0:c0 + cn],
                                         start=(d == 0), stop=(d == DT - 1))
                nc.scalar.activation(h1[:], p[:], AF.Relu, bias=bias["b1"][:], scale=1.0)
                y = acts.tile([128, DT, TT], dt.bfloat16, tag="q")
                for o in range(DT):
                    p = psums.tile([128, TT], dt.float32, tag="big")
                    for (c0, cn) in CHUNKS:
                        nc.tensor.matmul(p[:, c0:c0 + cn], w["w2"][:, 128 * o:128 * o + 128],
                                         h1[:, c0:c0 + cn], start=True, stop=False)
                        nc.tensor.matmul(p[:, c0:c0 + cn], smalls["ident"][:],
                                         rc[:, o, c0:c0 + cn], start=False, stop=True)
                    nc.scalar.activation(y[:, o], p[:], AF.Identity,
                                         bias=bias["b2"][:, o:o + 1], scale=1.0)

                # --- ln_out -> next cat (with affine go/bo2) ---
                cat_next = acts.tile([128, DT, TT], dt.bfloat16, tag=f"cat{(l + 1) % 2}")
                zo = acts.tile([128, DT, TT], dt.bfloat16, tag="z")
                _ln_normalize(nc, acts, sbufs, psums, smalls, y, zo, eps_tile)
                for d in range(DT):
                    nc.vector.tensor_scalar(cat_next[:, d], zo[:, d],
                                            bias["go"][:, d:d + 1], bias["bo2"][:, d:d + 1],
                                            ALU.mult, ALU.add)
                cat = cat_next

            # --- mean-pool utterance tokens (pos 1..4 of each 5-block) ---
            out_sb = sbufs.tile([128, DT], dt.float32, tag="outsb")
            for d in range(DT):
                view = cat[:, d, :].rearrange("p (s j) -> p s j", j=5)[:, :, 1:5]
                nc.vector.tensor_reduce(out_sb[:, d:d + 1], view,
                                        axis=mybir.AxisListType.XY, op=ALU.add)
            nc.vector.tensor_scalar_mul(out_sb[:], out_sb[:], 1.0 / U)
            nc.sync.dma_start(out_dram, out_sb[:])
    nc.compile()
    return nc


def _build():
    nc = bacc.Bacc("TRN2", target_bir_lowering=False, debug=False, num_devices=NCORES)
    return _trace(nc)


def kernel(**inputs):
    global _COMPILED
    ins = {k: np.asarray(v) for k, v in inputs.items()}
    shared = _host_prep(ins)
    idx = _tok_index()
    x = ins["x"].astype(np.float32)          # [B, T, D]
    xp = x[:, idx, :]                        # [B, TT, D]
    xT = np.ascontiguousarray(xp.transpose(0, 2, 1)).astype(bf16)  # [B, D, TT]
    if _COMPILED is None:
        _COMPILED = _build()
    nc = _COMPILED
    in_maps = []
    for b in range(NCORES):
        m = dict(shared)
        m["xT"] = xT[b]
        in_maps.append(m)
    res = bass_utils.run_bass_kernel_spmd(nc, in_maps, core_ids=list(range(NCORES)))
    outs = []
    for b in range(NCORES):
        o = res.results[b]["out"]            # [128, DT]
        outs.append(o.T.reshape(D))          # d = dtile*128 + p
    return np.stack(outs).astype(np.float32)



# revision 8
# speedup vs baseline: 1.2114x; 1.2114x over previous
"""Trainium2 Bass kernel for nn_BasicNet4 (Emformer encoder, sparse attention).

Strategy:
  - Data-parallel over batch B=8 across 8 NeuronCores (weights replicated).
  - Tokens reordered host-side into segment-interleaved order:
    seg i -> [rc_i, u_{4i}, u_{4i+1}, u_{4i+2}, u_{4i+3}]  (5 tokens x 256 segs = 1280)
    so attention is block-diagonal with 5x5 blocks.
  - Activations kept transposed in SBUF: [d on partitions (4 tiles of 128), tokens on free].
  - LayerNorm gains/biases folded into weights host-side; LN stats computed with
    ones-matmul partition reductions on the PE (broadcast form).
  - Attention masks folded into the score matmuls as extra low-rank (+/-C indicator)
    contraction terms; block-diagonal attention computed per 128-token diagonal tile
    plus small "halo" edge strips.
  - bf16 matmul operands / residual stream, fp32 PSUM accumulation.
"""

import sys

sys.path.insert(0, "/opt/trn_rl_repo")

import numpy as np
import ml_dtypes

import concourse.bass as bass
import concourse.mybir as mybir
import concourse.tile as tile
from concourse import bass_utils, bacc

bf16 = ml_dtypes.bfloat16
dt = mybir.dt
AF = mybir.ActivationFunctionType
ALU = mybir.AluOpType

# Model config (hardcoded from the problem spec)
D, H, FFN, L = 512, 4, 128, 4
SEG, RC = 4, 1
B, T = 8, 1025
U = T - RC            # 1024
NSEG = U // SEG       # 256
TT = NSEG * (SEG + RC)  # 1280 interleaved tokens
NT = TT // 128        # 10 token tiles
DT = D // 128         # 4 d tiles
DH = D // H           # 128 (= one partition tile per head)
NCORES = 8
CHUNKS = [(0, 512), (512, 512), (1024, 256)]  # free-dim chunks <= 512 (one PSUM bank)

CBF = np.float32(bf16(np.float32(1e9)))  # mask constant, exactly representable in bf16

_COMPILED = None


def _tok_index():
    # interleaved token t -> original frame index in x[:, :T]
    t = np.arange(TT)
    seg = t // 5
    pos = t % 5
    off = np.array([4, 0, 1, 2, 3])[pos]
    return 4 * seg + off  # in [0, 1024]


def _qt_geometry(qt):
    """MID window is the aligned [128qt, 128qt+128). LEFT/RIGHT edges are the
    few extra k-tokens of the straddling segments."""
    q0 = 128 * qt
    sk = 5 * (q0 // 5)
    op = q0 - sk                      # 0..4
    ek = min(5 * (-(-(q0 + 128) // 5)), TT)
    nL = op                           # left edge width (tokens [sk, q0))
    nR = max(ek - (q0 + 128), 0)      # right edge width (tokens [q0+128, ek))
    return q0, sk, op, nL, nR


def _mask_consts():
    """Per-qt mask matmul operands (host-computed, bf16).
    MID:  Lmid[qt] [128,128] (lhsT), Rmid[qt] [128,128] (rhs):
          sum_r Lmid[r,k]*Rmid[r,j] = -C + C*[seg(k)==seg(j)]  (window-local segs)
    EDGE: Lel[qt] [128,8], Rel[qt] [128,128]: same for the 8 edge slots
          (slots 0..3 = LEFT tokens, 4..7 = RIGHT tokens; invalid slots -> -C only).
    """
    Lmid = np.zeros((NT, 128, 128), np.float32)
    Rmid = np.zeros((NT, 128, 128), np.float32)
    Lel = np.zeros((NT, 128, 8), np.float32)
    Rel = np.zeros((NT, 128, 128), np.float32)
    for qt in range(NT):
        q0, sk, op, nL, nR = _qt_geometry(qt)
        segq = (op + np.arange(128)) // 5       # window-local seg of q (and mid k)
        # MID
        Lmid[qt, 0, :] = 1.0
        Rmid[qt, 0, :] = -CBF
        nseg = segq[-1] + 1
        for i in range(nseg):
            Lmid[qt, 1 + i, :] = (segq == i)
            Rmid[qt, 1 + i, :] = CBF * (segq == i)
        # EDGE
        Lel[qt, 0, :] = 1.0
        Rel[qt, 0, :] = -CBF
        slot_seg = np.full(8, -1)
        for s in range(nL):
            slot_seg[s] = 0                      # left tokens are in window-local seg 0
        for s in range(nR):
            slot_seg[4 + s] = (128 + op + s) // 5
        esegs = sorted(set(slot_seg[slot_seg >= 0]))
        for j, e in enumerate(esegs):
            Lel[qt, 1 + j, :] = (slot_seg == e)
            Rel[qt, 1 + j, :] = CBF * (segq == e)
    return Lmid.astype(bf16), Rmid.astype(bf16), Lel.astype(bf16), Rel.astype(bf16)


def _host_prep(ins):
    """Fold LN affines into weights, transpose, cast. Returns shared input map."""
    f32 = np.float32
    m = {}
    scale = np.float32(DH) ** -0.5
    for l in range(L):
        g_i, b_i = f32(ins["ln_in_g"][l]), f32(ins["ln_in_b"][l])
        g_f, b_f = f32(ins["ff_ln_g"][l]), f32(ins["ff_ln_b"][l])
        Wq = f32(ins["Wq"][l]);  bq = f32(ins["bq"][l])
        Wk = f32(ins["Wkv"][l][:D]);  bk = f32(ins["bkv"][l][:D])
        Wv = f32(ins["Wkv"][l][D:]);  bv = f32(ins["bkv"][l][D:])
        Wo = f32(ins["Wo"][l]);  bo = f32(ins["bo"][l])
        W1 = f32(ins["W1"][l]);  b1 = f32(ins["b1"][l])
        W2 = f32(ins["W2"][l]);  b2 = f32(ins["b2"][l])
        Wq_ = scale * (Wq * g_i[None, :]); bq_ = scale * (bq + Wq @ b_i)
        Wk_ = Wk * g_i[None, :];           bk_ = bk + Wk @ b_i
        Wv_ = Wv * g_i[None, :];           bv_ = bv + Wv @ b_i
        # softmax rows sum to 1, so attention(v + bv) = attention(v) + bv;
        # fold bv through Wo into bo and drop the on-device v-bias entirely.
        bo = bo + Wo @ bv_
        W1_ = W1 * g_f[None, :];           b1_ = b1 + W1 @ b_f
        m[f"wq{l}"] = Wq_.T.copy().astype(bf16)   # [din, dout]
        m[f"wk{l}"] = Wk_.T.copy().astype(bf16)
        m[f"wv{l}"] = Wv_.T.copy().astype(bf16)
        m[f"wo{l}"] = Wo.T.copy().astype(bf16)
        m[f"w1{l}"] = W1_.T.copy().astype(bf16)   # [512, 128]
        m[f"w2{l}"] = W2.T.copy().astype(bf16)    # [128, 512]
        m[f"bq{l}"] = bq_.reshape(DT, 128).T.copy()       # [128, DT] f32 per-partition
        m[f"bk{l}"] = bk_.reshape(DT, 128).T.copy()
        m[f"bo{l}"] = bo.reshape(DT, 128).T.copy()
        m[f"b1{l}"] = b1_.reshape(1, 128).T.copy()        # [128, 1]
        m[f"b2{l}"] = b2.reshape(DT, 128).T.copy()
        m[f"go{l}"] = f32(ins["ln_out_g"][l]).reshape(DT, 128).T.copy()
        m[f"bo2{l}"] = f32(ins["ln_out_b"][l]).reshape(DT, 128).T.copy()
    Lmid, Rmid, Lel, Rel = _mask_consts()
    m["lmid"] = np.ascontiguousarray(Lmid.transpose(1, 0, 2))  # [128, NT, 128]
    m["rmid"] = np.ascontiguousarray(Rmid.transpose(1, 0, 2))
    m["lel"] = np.ascontiguousarray(Lel.transpose(1, 0, 2))    # [128, NT, 8]
    m["rel"] = np.ascontiguousarray(Rel.transpose(1, 0, 2))
    m["ones_c"] = np.full((128, 128), 1.0 / D, bf16)           # stats lhsT (bcast reduce)
    m["allones"] = np.ones((128, 128), bf16)                   # denominator lhsT
    m["ones1"] = np.ones((1, 128), bf16)                       # K=1 bcast lhsT
    m["ident"] = np.eye(128, dtype=bf16)                       # residual adds
    return m


def _dram_inputs(nc):
    a = {}
    def inp(name, shape, dtype):
        a[name] = nc.dram_tensor(name, list(shape), dtype, kind="ExternalInput").ap()
    inp("xT", (D, TT), dt.bfloat16)
    for l in range(L):
        inp(f"wq{l}", (D, D), dt.bfloat16); inp(f"wk{l}", (D, D), dt.bfloat16)
        inp(f"wv{l}", (D, D), dt.bfloat16); inp(f"wo{l}", (D, D), dt.bfloat16)
        inp(f"w1{l}", (D, FFN), dt.bfloat16); inp(f"w2{l}", (FFN, D), dt.bfloat16)
        inp(f"bq{l}", (128, DT), dt.float32); inp(f"bk{l}", (128, DT), dt.float32)
        inp(f"bo{l}", (128, DT), dt.float32)
        inp(f"b1{l}", (128, 1), dt.float32); inp(f"b2{l}", (128, DT), dt.float32)
        inp(f"go{l}", (128, DT), dt.float32); inp(f"bo2{l}", (128, DT), dt.float32)
    inp("lmid", (128, NT, 128), dt.bfloat16); inp("rmid", (128, NT, 128), dt.bfloat16)
    inp("lel", (128, NT, 8), dt.bfloat16); inp("rel", (128, NT, 128), dt.bfloat16)
    inp("ones_c", (128, 128), dt.bfloat16); inp("allones", (128, 128), dt.bfloat16)
    inp("ones1", (1, 128), dt.bfloat16); inp("ident", (128, 128), dt.bfloat16)
    out = nc.dram_tensor("out", [128, DT], dt.float32, kind="ExternalOutput").ap()
    return a, out


def _ln_normalize(nc, acts, sbufs, psums, smalls, cat, z_out, eps_tile):
    """z = (cat - mean) * rstd in bcast form. cat/z: [128, DT, TT] bf16 sbuf."""
    ones_c = smalls["ones_c"]
    # squares on GPSIMD (bf16)
    sq = acts.tile([128, DT, TT], dt.bfloat16, tag="sq")
    for d in range(DT):
        nc.gpsimd.tensor_tensor(sq[:, d], cat[:, d], cat[:, d], ALU.mult)
    p_mu = psums.tile([128, TT], dt.float32, tag="big")
    p_e2 = psums.tile([128, TT], dt.float32, tag="big")
    for (c0, cn) in CHUNKS:
        for d in range(DT):
            nc.tensor.matmul(p_mu[:, c0:c0 + cn], ones_c[:], cat[:, d, c0:c0 + cn],
                             start=(d == 0), stop=(d == DT - 1))
        for d in range(DT):
            nc.tensor.matmul(p_e2[:, c0:c0 + cn], ones_c[:], sq[:, d, c0:c0 + cn],
                             start=(d == 0), stop=(d == DT - 1))
    # Note ones_c = 1/512 -> p_mu = mean, p_e2 = E[x^2]; all rows identical (allones trick
    # not needed: ones_c is [128,1] so out partition count is... see below)
    mu_b = sbufs.tile([128, TT], dt.bfloat16, tag="mu")
    sq_mu = sbufs.tile([128, TT], dt.float32, tag="sqmu")
    var = sbufs.tile([128, TT], dt.float32, tag="var")
    A = sbufs.tile([128, TT], dt.float32, tag="A")
    nc.scalar.activation(sq_mu[:], p_mu[:], AF.Square)
    nc.vector.tensor_copy(mu_b[:], p_mu[:])
    nc.vector.tensor_tensor(var[:], p_e2[:], sq_mu[:], ALU.subtract)
    nc.scalar.activation(var[:], var[:], AF.Sqrt, bias=eps_tile[:], scale=1.0)
    nc.vector.reciprocal_approx_fast(A[:], var[:])
    for d in range(DT):
        z1 = sbufs.tile([128, TT], dt.bfloat16, tag="z1")
        nc.vector.tensor_tensor(z1[:], cat[:, d], mu_b[:], ALU.subtract)
        nc.vector.tensor_tensor(z_out[:, d], z1[:], A[:], ALU.mult)


def _trace(nc):
    a, out_dram = _dram_inputs(nc)
    with tile.TileContext(nc) as tc:
        import contextlib
        ctx = contextlib.ExitStack()
        with ctx:
            consts = ctx.enter_context(tc.tile_pool(name="consts", bufs=1))
            wpool = ctx.enter_context(tc.tile_pool(name="w", bufs=2))
            acts = ctx.enter_context(tc.tile_pool(name="acts", bufs=1))
            sbufs = ctx.enter_context(tc.tile_pool(name="sbufs", bufs=2))
            psums = ctx.enter_context(tc.tile_pool(name="psums", bufs=2, space="PSUM"))
            pv = ctx.enter_context(tc.tile_pool(name="pv", bufs=2, space="PSUM"))

            # constants
            smalls = {}
            for name, shape, dd in [
                ("lmid", [128, NT, 128], dt.bfloat16), ("rmid", [128, NT, 128], dt.bfloat16),
                ("lel", [128, NT, 8], dt.bfloat16), ("rel", [128, NT, 128], dt.bfloat16),
                ("ones_c", [128, 128], dt.bfloat16), ("allones", [128, 128], dt.bfloat16),
                ("ones1", [1, 128], dt.bfloat16), ("ident", [128, 128], dt.bfloat16),
            ]:
                t = consts.tile(shape, dd, tag=name)
                nc.sync.dma_start(t[:], a[name])
                smalls[name] = t
            eps_tile = consts.tile([128, 1], dt.float32)
            nc.vector.memset(eps_tile[:], 1e-5)

            # initial residual stream (transposed, interleaved)
            cat = acts.tile([128, DT, TT], dt.bfloat16, tag="cat0")
            nc.sync.dma_start(cat[:], a["xT"].rearrange("(dtile p) t -> p dtile t", p=128))

            for l in range(L):
                # --- load layer weights ---
                w = {}
                for nm, shape in [("wq", [128, DT, D]), ("wk", [128, DT, D]),
                                  ("wv", [128, DT, D]), ("wo", [128, DT, D]),
                                  ("w1", [128, DT, FFN]), ("w2", [128, D])]:
                    t = wpool.tile(shape, dt.bfloat16, tag=nm)
                    src = a[f"{nm}{l}"]
                    if nm == "w2":
                        nc.sync.dma_start(t[:], src)
                    else:
                        nc.sync.dma_start(t[:], src.rearrange("(dtile p) o -> p dtile o", p=128))
                    w[nm] = t
                bias = {}
                for nm in ["bq", "bk", "bo", "b1", "b2", "go", "bo2"]:
                    t = wpool.tile([128, DT] if nm != "b1" else [128, 1], dt.float32, tag=nm)
                    nc.sync.dma_start(t[:], a[f"{nm}{l}"])
                    bias[nm] = t
                # --- ln_in -> z ---
                z = acts.tile([128, DT, TT], dt.bfloat16, tag="z")
                _ln_normalize(nc, acts, sbufs, psums, smalls, cat, z, eps_tile)

                # --- Q, K projections (weights stationary -> transposed out) ---
                qk = {}
                for nm, bnm in [("wq", "bq"), ("wk", "bk")]:
                    dst = acts.tile([128, DT, TT], dt.bfloat16, tag="q" if nm == "wq" else "k")
                    for o in range(DT):
                        p = psums.tile([128, TT], dt.float32, tag="big")
                        for (c0, cn) in CHUNKS:
                            for d in range(DT):
                                nc.tensor.matmul(
                                    p[:, c0:c0 + cn],
                                    w[nm][:, d, 128 * o:128 * o + 128],
                                    z[:, d, c0:c0 + cn],
                                    start=(d == 0), stop=(d == DT - 1))
                        nc.scalar.activation(dst[:, o], p[:], AF.Identity,
                                             bias=bias[bnm][:, o:o + 1], scale=1.0)
                    qk[nm] = dst
                q_t, k_t = qk["wq"], qk["wk"]

                # --- V projection (acts stationary -> natural out [t, d]) ---
                v_nat = acts.tile([128, NT, D], dt.bfloat16, tag="v")
                for tt_i in range(NT):
                    p = pv.tile([128, D], dt.float32, tag="small")
                    for d in range(DT):
                        nc.tensor.matmul(p[:], z[:, d, 128 * tt_i:128 * tt_i + 128],
                                         w["wv"][:, d, :], start=(d == 0),
                                         stop=(d == DT - 1))
                    nc.vector.tensor_copy(v_nat[:, tt_i], p[:])

                # --- halos (K columns easy; V rows via partition-offset copies) ---
                k_halo = sbufs.tile([128, DT, NT, 8], dt.bfloat16, tag="khalo")
                v_halo = acts.tile([8, NT, D], dt.bfloat16, tag="vhalo")
                nc.gpsimd.memset(k_halo[:], 0.0)
                nc.gpsimd.memset(v_halo[:], 0.0)
                for qt in range(NT):
                    q0, sk, op, nL, nR = _qt_geometry(qt)
                    if nL > 0:
                        nc.gpsimd.tensor_copy(k_halo[:, :, qt, 0:nL], k_t[:, :, sk:sk + nL])
                        nc.sync.dma_start(v_halo[0:nL, qt, :],
                                          v_nat[128 - nL:128, qt - 1, :])
                    if nR > 0:
                        nc.gpsimd.tensor_copy(k_halo[:, :, qt, 4:4 + nR],
                                              k_t[:, :, q0 + 128:q0 + 128 + nR])
                        nc.sync.dma_start(v_halo[4:4 + nR, qt, :],
                                          v_nat[0:nR, qt + 1, :])

                # --- attention per head ---
                attn = acts.tile([128, DT, TT], dt.bfloat16, tag="attn")
                for h in range(H):
                    p_mid = psums.tile([128, TT], dt.float32, tag="big")
                    p_edge = psums.tile([128, TT], dt.float32, tag="big")
                    for qt in range(NT):
                        q0 = 128 * qt
                        qs = q_t[:, h, q0:q0 + 128]
                        nc.tensor.matmul(p_mid[:, q0:q0 + 128], smalls["lmid"][:, qt],
                                         smalls["rmid"][:, qt], start=True, stop=False)
                        nc.tensor.matmul(p_mid[:, q0:q0 + 128], k_t[:, h, q0:q0 + 128],
                                         qs, start=False, stop=True)
                        nc.tensor.matmul(p_edge[0:8, q0:q0 + 128], smalls["lel"][:, qt],
                                         smalls["rel"][:, qt], start=True, stop=False)
                        nc.tensor.matmul(p_edge[0:8, q0:q0 + 128], k_halo[:, h, qt], qs,
                                         start=False, stop=True)
                    pa = sbufs.tile([128, TT], dt.bfloat16, tag="pa")
                    pe = sbufs.tile([8, TT], dt.bfloat16, tag="pe")
                    nc.scalar.activation(pa[:], p_mid[:], AF.Exp)
                    nc.scalar.activation(pe[:], p_edge[0:8, :], AF.Exp)
                    # denominator (broadcast over partitions via all-ones lhsT)
                    p_den = psums.tile([128, TT], dt.float32, tag="big")
                    for (c0, cn) in CHUNKS:
                        nc.tensor.matmul(p_den[:, c0:c0 + cn], smalls["allones"][:],
                                         pa[:, c0:c0 + cn], start=True, stop=False)
                        nc.tensor.matmul(p_den[:, c0:c0 + cn], smalls["allones"][0:8],
                                         pe[:, c0:c0 + cn], start=False, stop=True)
                    rec = sbufs.tile([128, TT], dt.float32, tag="rec")
                    nc.vector.reciprocal_approx_fast(rec[:], p_den[:])
                    # attn value matmuls
                    p_av = psums.tile([128, TT], dt.float32, tag="big")
                    for qt in range(NT):
                        q0 = 128 * qt
                        nc.tensor.matmul(p_av[:, q0:q0 + 128], v_nat[:, qt, 128 * h:128 * h + 128],
                                         pa[:, q0:q0 + 128], start=True, stop=False)
                        nc.tensor.matmul(p_av[:, q0:q0 + 128], v_halo[:, qt, 128 * h:128 * h + 128],
                                         pe[:, q0:q0 + 128], start=False, stop=True)
                    nc.vector.tensor_tensor(attn[:, h], p_av[:], rec[:], ALU.mult)

                # --- Wo projection + residual ---
                rc = acts.tile([128, DT, TT], dt.bfloat16, tag=f"cat{(l + 1) % 2}")
                for o in range(DT):
                    p = psums.tile([128, TT], dt.float32, tag="big")
                    for (c0, cn) in CHUNKS:
                        for d in range(DT):
                            nc.tensor.matmul(p[:, c0:c0 + cn],
                                             w["wo"][:, d, 128 * o:128 * o + 128],
                                             attn[:, d, c0:c0 + cn],
                                             start=(d == 0), stop=False)
                        nc.tensor.matmul(p[:, c0:c0 + cn], smalls["ident"][:],
                                         cat[:, o, c0:c0 + cn], start=False, stop=True)
                    nc.scalar.activation(rc[:, o], p[:], AF.Identity,
                                         bias=bias["bo"][:, o:o + 1], scale=1.0)

                # --- ff_ln -> zf ---
                zf = acts.tile([128, DT, TT], dt.bfloat16, tag="z")
                _ln_normalize(nc, acts, sbufs, psums, smalls, rc, zf, eps_tile)

                # --- FFN ---
                h1 = acts.tile([128, TT], dt.bfloat16, tag="h1")
                p = psums.tile([128, TT], dt.float32, tag="big")
                for (c0, cn) in CHUNKS:
                    for d in range(DT):
                        nc.tensor.matmul(p[:, c0:c0 + cn], w["w1"][:, d, :],
                                         zf[:, d, c0:c0 + cn],
                                         start=(d == 0), stop=(d == DT - 1))
                nc.scalar.activation(h1[:], p[:], AF.Relu, bias=bias["b1"][:], scale=1.0)
                y = acts.tile([128, DT, TT], dt.bfloat16, tag="q")
                for o in range(DT):
                    p = psums.tile([128, TT], dt.float32, tag="big")
                    for (c0, cn) in CHUNKS:
                        nc.tensor.matmul(p[:, c0:c0 + cn], w["w2"][:, 128 * o:128 * o + 128],
                                         h1[:, c0:c0 + cn], start=True, stop=False)
                        nc.tensor.matmul(p[:, c0:c0 + cn], smalls["ident"][:],
                                         rc[:, o, c0:c0 + cn], start=False, stop=True)
                    nc.scalar.activation(y[:, o], p[:], AF.Identity,
                                         bias=bias["b2"][:, o:o + 1], scale=1.0)

                # --- ln_out -> next cat (with affine go/bo2) ---
                cat_next = acts.tile([128, DT, TT], dt.bfloat16, tag=f"cat{(l + 1) % 2}")
                zo = acts.tile([128, DT, TT], dt.bfloat16, tag="z")
                _ln_normalize(nc, acts, sbufs, psums, smalls, y, zo, eps_tile)
                for d in range(DT):
                    nc.vector.tensor_scalar(cat_next[:, d], zo[:, d],
                                            bias["go"][:, d:d + 1], bias["bo2"][:, d:d + 1],
                                            ALU.mult, ALU.add)
                cat = cat_next

            # --- mean-pool utterance tokens (pos 1..4 of each 5-block) ---
            out_sb = sbufs.tile([128, DT], dt.float32, tag="outsb")
            for d in range(DT):
                view = cat[:, d, :].rearrange("p (s j) -> p s j", j=5)[:, :, 1:5]
                nc.vector.tensor_reduce(out_sb[:, d:d + 1], view,
                                        axis=mybir.AxisListType.XY, op=ALU.add)
            nc.vector.tensor_scalar_mul(out_sb[:], out_sb[:], 1.0 / U)
            nc.sync.dma_start(out_dram, out_sb[:])
    nc.compile()
    return nc


def _build():
    nc = bacc.Bacc("TRN2", target_bir_lowering=False, debug=False, num_devices=NCORES)
    return _trace(nc)


def kernel(**inputs):
    global _COMPILED
    ins = {k: np.asarray(v) for k, v in inputs.items()}
    shared = _host_prep(ins)
    idx = _tok_index()
    x = ins["x"].astype(np.float32)          # [B, T, D]
    xp = x[:, idx, :]                        # [B, TT, D]
    xT = np.ascontiguousarray(xp.transpose(0, 2, 1)).astype(bf16)  # [B, D, TT]
    if _COMPILED is None:
        _COMPILED = _build()
    nc = _COMPILED
    in_maps = []
    for b in range(NCORES):
        m = dict(shared)
        m["xT"] = xT[b]
        in_maps.append(m)
    res = bass_utils.run_bass_kernel_spmd(nc, in_maps, core_ids=list(range(NCORES)))
    outs = []
    for b in range(NCORES):
        o = res.results[b]["out"]            # [128, DT]
        outs.append(o.T.reshape(D))          # d = dtile*128 + p
    return np.stack(outs).astype(np.float32)



# revision 15
# speedup vs baseline: 1.4775x; 1.2197x over previous
"""Trainium2 Bass kernel for nn_BasicNet4 (Emformer encoder, sparse attention).

Strategy:
  - Data-parallel over batch B=8 across 8 NeuronCores (weights replicated).
  - Tokens reordered host-side into segment-interleaved order:
    seg i -> [rc_i, u_{4i}, u_{4i+1}, u_{4i+2}, u_{4i+3}]  (5 tokens x 256 segs = 1280)
    so attention is block-diagonal with 5x5 blocks.
  - Attention computed in 125-token windows (25 whole segments) so every
    window has the SAME block-diagonal mask (rank-26 factorization folded
    into the scores matmul) and there are no cross-window halos/edges.
  - Activations kept transposed in SBUF: [d on partitions (4 tiles of 128), tokens on free].
  - LayerNorm gains/biases folded into weights host-side; LN stats computed with
    ones-matmul partition reductions on the PE (broadcast form), variance finished
    in-PSUM via a K=1 matmul of -mu^2, and rstd = exp(-0.5*ln(var+eps)) on the
    scalar engine (no reciprocal, no sqrt table set).
  - V bias folded through Wo into bo host-side (softmax rows sum to 1).
  - All stages chunked (512-col chunks / per-window) with small rotating
    PSUM tags so the tile scheduler can pipeline engines 2-deep.
  - bf16 matmul operands / residual stream, fp32 PSUM accumulation.
"""

import sys

sys.path.insert(0, "/opt/trn_rl_repo")

import numpy as np
import ml_dtypes

import concourse.bass as bass
import concourse.mybir as mybir
import concourse.tile as tile
from concourse import bass_utils, bacc

bf16 = ml_dtypes.bfloat16
dt = mybir.dt
AF = mybir.ActivationFunctionType
ALU = mybir.AluOpType

# Model config (hardcoded from the problem spec)
D, H, FFN, L = 512, 4, 128, 4
SEG, RC = 4, 1
B, T = 8, 1025
U = T - RC            # 1024
NSEG = U // SEG       # 256
TT = NSEG * (SEG + RC)  # 1280 interleaved tokens
DT = D // 128         # 4 d tiles
DH = D // H           # 128 (= one partition tile per head)
NCORES = 8
CHUNKS = [(0, 512), (512, 512), (1024, 256)]  # free-dim chunks <= 512 (one PSUM bank)

W = 125               # attention window = 25 whole segments
NWF = TT // W         # 10 full windows
WL = TT - NWF * W     # 30-token last window (6 segments)
WINDOWS = [(w * W, W) for w in range(NWF)] + [(NWF * W, WL)]
NW = len(WINDOWS)     # 11

CBF = np.float32(bf16(np.float32(1e9)))  # mask constant, exactly representable in bf16

_COMPILED = None


def _tok_index():
    # interleaved token t -> original frame index in x[:, :T]
    t = np.arange(TT)
    seg = t // 5
    pos = t % 5
    off = np.array([4, 0, 1, 2, 3])[pos]
    return 4 * seg + off  # in [0, 1024]


def _mask_consts():
    """Uniform per-window mask factorization.
    Lm [26, W] (lhsT), Rm [26, W] (rhs): sum_r Lm[r,k]*Rm[r,j] =
    -C + C*[seg(k)==seg(j)] for window-local tokens. Every 125-token window
    holds exactly 25 segments, so one constant serves all full windows; the
    30-token last window uses the top-left slice with 6+1 rows.
    """
    nseg = W // 5
    seg = np.arange(W) // 5
    Lm = np.zeros((1 + nseg, W), np.float32)
    Rm = np.zeros((1 + nseg, W), np.float32)
    Lm[0, :] = 1.0
    Rm[0, :] = -CBF
    for i in range(nseg):
        Lm[1 + i] = (seg == i)
        Rm[1 + i] = CBF * (seg == i)
    return Lm.astype(bf16), Rm.astype(bf16)


def _host_prep(ins):
    """Fold LN affines into weights, transpose, cast. Returns shared input map."""
    f32 = np.float32
    m = {}
    scale = np.float32(DH) ** -0.5
    for l in range(L):
        g_i, b_i = f32(ins["ln_in_g"][l]), f32(ins["ln_in_b"][l])
        g_f, b_f = f32(ins["ff_ln_g"][l]), f32(ins["ff_ln_b"][l])
        Wq = f32(ins["Wq"][l]);  bq = f32(ins["bq"][l])
        Wk = f32(ins["Wkv"][l][:D]);  bk = f32(ins["bkv"][l][:D])
        Wv = f32(ins["Wkv"][l][D:]);  bv = f32(ins["bkv"][l][D:])
        Wo = f32(ins["Wo"][l]);  bo = f32(ins["bo"][l])
        W1 = f32(ins["W1"][l]);  b1 = f32(ins["b1"][l])
        W2 = f32(ins["W2"][l]);  b2 = f32(ins["b2"][l])
        Wq_ = scale * (Wq * g_i[None, :]); bq_ = scale * (bq + Wq @ b_i)
        Wk_ = Wk * g_i[None, :];           bk_ = bk + Wk @ b_i
        Wv_ = Wv * g_i[None, :];           bv_ = bv + Wv @ b_i
        # softmax rows sum to 1, so attention(v + bv) = attention(v) + bv;
        # fold bv through Wo into bo and drop the on-device v-bias entirely.
        bo = bo + Wo @ bv_
        W1_ = W1 * g_f[None, :];           b1_ = b1 + W1 @ b_f
        m[f"wq{l}"] = Wq_.T.copy().astype(bf16)   # [din, dout]
        m[f"wk{l}"] = Wk_.T.copy().astype(bf16)
        m[f"wv{l}"] = Wv_.T.copy().astype(bf16)
        m[f"wo{l}"] = Wo.T.copy().astype(bf16)
        m[f"w1{l}"] = W1_.T.copy().astype(bf16)   # [512, 128]
        m[f"w2{l}"] = W2.T.copy().astype(bf16)    # [128, 512]
        m[f"bq{l}"] = bq_.reshape(DT, 128).T.copy()       # [128, DT] f32 per-partition
        m[f"bk{l}"] = bk_.reshape(DT, 128).T.copy()
        m[f"bo{l}"] = bo.reshape(DT, 128).T.copy()
        m[f"b1{l}"] = b1_.reshape(1, 128).T.copy()        # [128, 1]
        m[f"b2{l}"] = b2.reshape(DT, 128).T.copy()
        m[f"go{l}"] = f32(ins["ln_out_g"][l]).reshape(DT, 128).T.copy()
        m[f"bo2{l}"] = f32(ins["ln_out_b"][l]).reshape(DT, 128).T.copy()
    Lm, Rm = _mask_consts()
    m["lmask"] = np.ascontiguousarray(Lm)                  # [26, W]
    m["rmask"] = np.ascontiguousarray(Rm)                  # [26, W]
    m["ones_c"] = np.full((128, 128), 1.0 / D, bf16)       # stats lhsT (bcast reduce)
    m["allones"] = np.ones((128, 128), bf16)               # denominator lhsT
    m["onesneg"] = np.full((1, 128), -1.0, bf16)           # K=1 -mu^2 accumulation lhsT
    m["ident"] = np.eye(128, dtype=bf16)                   # residual adds
    return m


def _dram_inputs(nc):
    a = {}
    def inp(name, shape, dtype):
        a[name] = nc.dram_tensor(name, list(shape), dtype, kind="ExternalInput").ap()
    inp("xT", (D, TT), dt.bfloat16)
    for l in range(L):
        inp(f"wq{l}", (D, D), dt.bfloat16); inp(f"wk{l}", (D, D), dt.bfloat16)
        inp(f"wv{l}", (D, D), dt.bfloat16); inp(f"wo{l}", (D, D), dt.bfloat16)
        inp(f"w1{l}", (D, FFN), dt.bfloat16); inp(f"w2{l}", (FFN, D), dt.bfloat16)
        inp(f"bq{l}", (128, DT), dt.float32); inp(f"bk{l}", (128, DT), dt.float32)
        inp(f"bo{l}", (128, DT), dt.float32)
        inp(f"b1{l}", (128, 1), dt.float32); inp(f"b2{l}", (128, DT), dt.float32)
        inp(f"go{l}", (128, DT), dt.float32); inp(f"bo2{l}", (128, DT), dt.float32)
    inp("lmask", (26, W), dt.bfloat16); inp("rmask", (26, W), dt.bfloat16)
    inp("ones_c", (128, 128), dt.bfloat16); inp("allones", (128, 128), dt.bfloat16)
    inp("onesneg", (1, 128), dt.bfloat16); inp("ident", (128, 128), dt.bfloat16)
    out = nc.dram_tensor("out", [128, DT], dt.float32, kind="ExternalOutput").ap()
    return a, out


def _ln(nc, sbufs, psums, smalls, eps_tile, src, dst, affine=None):
    """dst = (src - mean) * rstd [* go + bo2], chunked over tokens.
    src/dst: [128, DT, TT] bf16 sbuf. Stats per chunk: mu and E[x^2] via
    ones-matmuls; -mu^2 accumulated into the E[x^2] psum by a K=1 matmul so
    the psum holds var; rstd = exp(-0.5*ln(var+eps)) on the scalar engine."""
    ones_c, onesneg = smalls["ones_c"], smalls["onesneg"]
    for ci, (c0, cn) in enumerate(CHUNKS):
        sqs = []
        for d in range(DT):
            sqt = sbufs.tile([128, 512], dt.bfloat16, tag=f"sq{d % 2}", bufs=2)
            eng = (nc.vector, nc.gpsimd, nc.vector, nc.gpsimd)[d]
            eng.tensor_tensor(sqt[:, :cn], src[:, d, c0:c0 + cn],
                              src[:, d, c0:c0 + cn], ALU.mult)
            sqs.append(sqt)
        p_mu = psums.tile([128, 512], dt.float32, tag="stA", bufs=2)
        p_e2 = psums.tile([128, 512], dt.float32, tag="stB", bufs=2)
        for d in range(DT):
            nc.tensor.matmul(p_mu[:, :cn], ones_c[:], src[:, d, c0:c0 + cn],
                             start=(d == 0), stop=(d == DT - 1))
        for d in range(DT):
            nc.tensor.matmul(p_e2[:, :cn], ones_c[:], sqs[d][:, :cn],
                             start=(d == 0), stop=False)
        mu_b = sbufs.tile([128, 512], dt.bfloat16, tag="mu", bufs=2)
        nc.vector.tensor_copy(mu_b[:, :cn], p_mu[:, :cn])
        musq = sbufs.tile([1, 512], dt.bfloat16, tag="musq", bufs=2)
        nc.vector.tensor_tensor(musq[0:1, :cn], mu_b[0:1, :cn], mu_b[0:1, :cn],
                                ALU.mult)
        nc.tensor.matmul(p_e2[:, :cn], onesneg[:], musq[0:1, :cn],
                         start=False, stop=True)   # p_e2 now holds var
        lnv = sbufs.tile([128, 512], dt.float32, tag="lnv", bufs=2)
        nc.scalar.activation(lnv[:, :cn], p_e2[:, :cn], AF.Ln,
                             bias=eps_tile[:], scale=1.0)
        A = sbufs.tile([128, 512], dt.bfloat16, tag="A", bufs=2)
        nc.scalar.activation(A[:, :cn], lnv[:, :cn], AF.Exp, bias=0.0, scale=-0.5)
        for d in range(DT):
            z1 = sbufs.tile([128, 512], dt.bfloat16, tag=f"z1{d % 2}", bufs=2)
            e1 = ((nc.vector, nc.gpsimd, nc.gpsimd, nc.vector)
                  if affine is not None else
                  (nc.vector, nc.gpsimd, nc.vector, nc.vector))[d]
            e1.tensor_tensor(z1[:, :cn], src[:, d, c0:c0 + cn], mu_b[:, :cn],
                             ALU.subtract)
            if affine is None:
                e2 = (nc.vector, nc.vector, nc.gpsimd, nc.vector)[d]
                e2.tensor_tensor(dst[:, d, c0:c0 + cn], z1[:, :cn], A[:, :cn],
                                 ALU.mult)
            else:
                go, bo2 = affine
                nc.vector.scalar_tensor_tensor(dst[:, d, c0:c0 + cn], z1[:, :cn],
                                               go[:, d:d + 1], A[:, :cn],
                                               ALU.mult, ALU.mult)
                nc.vector.tensor_scalar_add(dst[:, d, c0:c0 + cn],
                                            dst[:, d, c0:c0 + cn],
                                            bo2[:, d:d + 1])


def _trace(nc):
    a, out_dram = _dram_inputs(nc)
    with tile.TileContext(nc) as tc:
        import contextlib
        ctx = contextlib.ExitStack()
        with ctx:
            consts = ctx.enter_context(tc.tile_pool(name="consts", bufs=1))
            wpool = ctx.enter_context(tc.tile_pool(name="w", bufs=2))
            acts = ctx.enter_context(tc.tile_pool(name="acts", bufs=1))
            sbufs = ctx.enter_context(tc.tile_pool(name="sbufs", bufs=2))
            psums = ctx.enter_context(tc.tile_pool(name="psums", bufs=2, space="PSUM"))

            # constants
            smalls = {}
            for name, shape, dd in [
                ("lmask", [26, W], dt.bfloat16), ("rmask", [26, W], dt.bfloat16),
                ("ones_c", [128, 128], dt.bfloat16), ("allones", [128, 128], dt.bfloat16),
                ("onesneg", [1, 128], dt.bfloat16), ("ident", [128, 128], dt.bfloat16),
            ]:
                t = consts.tile(shape, dd, tag=name)
                nc.sync.dma_start(t[:], a[name])
                smalls[name] = t
            eps_tile = consts.tile([128, 1], dt.float32)
            nc.vector.memset(eps_tile[:], 1e-5)

            # initial residual stream (transposed, interleaved)
            cat = acts.tile([128, DT, TT], dt.bfloat16, tag="cat0")
            nc.sync.dma_start(cat[:], a["xT"].rearrange("(dtile p) t -> p dtile t", p=128))

            for l in range(L):
                # --- load layer weights ---
                w = {}
                for nm, shape in [("wq", [128, DT, D]), ("wk", [128, DT, D]),
                                  ("wv", [128, DT, D]), ("wo", [128, DT, D]),
                                  ("w1", [128, DT, FFN]), ("w2", [128, D])]:
                    t = wpool.tile(shape, dt.bfloat16, tag=nm)
                    src = a[f"{nm}{l}"]
                    if nm == "w2":
                        nc.sync.dma_start(t[:], src)
                    else:
                        nc.sync.dma_start(t[:], src.rearrange("(dtile p) o -> p dtile o", p=128))
                    w[nm] = t
                bias = {}
                for nm in ["bq", "bk", "bo", "b1", "b2", "go", "bo2"]:
                    t = wpool.tile([128, DT] if nm != "b1" else [128, 1], dt.float32, tag=nm)
                    nc.sync.dma_start(t[:], a[f"{nm}{l}"])
                    bias[nm] = t

                # --- ln_in -> z ---
                z = acts.tile([128, DT, TT], dt.bfloat16, tag="z")
                _ln(nc, sbufs, psums, smalls, eps_tile, cat, z)

                # --- Q, K projections (weights stationary -> transposed out) ---
                qk = {}
                for nm, bnm in [("wq", "bq"), ("wk", "bk")]:
                    dst = acts.tile([128, DT, TT], dt.bfloat16, tag="q" if nm == "wq" else "k")
                    for (c0, cn) in CHUNKS:
                        for o in range(DT):
                            p = psums.tile([128, 512], dt.float32, tag="pA", bufs=2)
                            for d in range(DT):
                                nc.tensor.matmul(
                                    p[:, :cn],
                                    w[nm][:, d, 128 * o:128 * o + 128],
                                    z[:, d, c0:c0 + cn],
                                    start=(d == 0), stop=(d == DT - 1))
                            if nm == "wq":
                                nc.scalar.activation(dst[:, o, c0:c0 + cn], p[:, :cn],
                                                     AF.Identity,
                                                     bias=bias[bnm][:, o:o + 1], scale=1.0)
                            else:
                                nc.vector.tensor_scalar_add(dst[:, o, c0:c0 + cn],
                                                            p[:, :cn],
                                                            bias[bnm][:, o:o + 1])
                    qk[nm] = dst
                q_t, k_t = qk["wq"], qk["wk"]

                # --- V projection (acts stationary -> natural out per window) ---
                v_nat = acts.tile([128, NW, D], dt.bfloat16, tag="v")
                for wi, (w0, wn) in enumerate(WINDOWS):
                    p = psums.tile([128, 512], dt.float32, tag="pB", bufs=2)
                    for d in range(DT):
                        nc.tensor.matmul(p[0:wn, :], z[:, d, w0:w0 + wn],
                                         w["wv"][:, d, :], start=(d == 0),
                                         stop=(d == DT - 1))
                    nc.vector.tensor_copy(v_nat[0:wn, wi], p[0:wn, :])

                # --- attention per 125-token window (all heads share psums) ---
                attn = acts.tile([128, DT, TT], dt.bfloat16, tag="attn")
                lmask, rmask, allones = smalls["lmask"], smalls["rmask"], smalls["allones"]
                nsg = 1 + W // 5
                nsgl = 1 + WL // 5
                for wi, (w0, wn) in enumerate(WINDOWS):
                    nr = nsg if wn == W else nsgl
                    sc = psums.tile([128, H, W], dt.float32, tag="stA", bufs=2)
                    for h in range(H):
                        nc.tensor.matmul(sc[0:wn, h, 0:wn], lmask[0:nr, 0:wn],
                                         rmask[0:nr, 0:wn], start=True, stop=False)
                        nc.tensor.matmul(sc[0:wn, h, 0:wn], k_t[:, h, w0:w0 + wn],
                                         q_t[:, h, w0:w0 + wn], start=False, stop=True)
                    pa = sbufs.tile([128, H, W], dt.bfloat16, tag="pa", bufs=2)
                    nc.scalar.activation(pa[0:wn, :, 0:wn], sc[0:wn, :, 0:wn], AF.Exp)
                    den = psums.tile([128, H, W], dt.float32, tag="stB", bufs=2)
                    p_av = psums.tile([128, H, W], dt.float32, tag="pA", bufs=2)
                    for h in range(H):
                        nc.tensor.matmul(den[:, h, 0:wn], allones[0:wn, :],
                                         pa[0:wn, h, 0:wn], start=True, stop=True)
                        nc.tensor.matmul(p_av[:, h, 0:wn],
                                         v_nat[0:wn, wi, 128 * h:128 * h + 128],
                                         pa[0:wn, h, 0:wn], start=True, stop=True)
                    rec = sbufs.tile([128, H, W], dt.float32, tag="rec", bufs=2)
                    nc.vector.reciprocal_approx_fast(rec[:, :, 0:wn], den[:, :, 0:wn])
                    nc.vector.tensor_tensor(attn[:, :, w0:w0 + wn], p_av[:, :, 0:wn],
                                            rec[:, :, 0:wn], ALU.mult)

                # --- Wo projection + residual ---
                rc = acts.tile([128, DT, TT], dt.bfloat16, tag=f"cat{(l + 1) % 2}")
                for (c0, cn) in CHUNKS:
                    for o in range(DT):
                        p = psums.tile([128, 512], dt.float32, tag="pA", bufs=2)
                        for d in range(DT):
                            nc.tensor.matmul(p[:, :cn],
                                             w["wo"][:, d, 128 * o:128 * o + 128],
                                             attn[:, d, c0:c0 + cn],
                                             start=(d == 0), stop=False)
                        nc.tensor.matmul(p[:, :cn], smalls["ident"][:],
                                         cat[:, o, c0:c0 + cn], start=False, stop=True)
                        nc.scalar.activation(rc[:, o, c0:c0 + cn], p[:, :cn], AF.Identity,
                                             bias=bias["bo"][:, o:o + 1], scale=1.0)

                # --- ff_ln -> zf ---
                zf = acts.tile([128, DT, TT], dt.bfloat16, tag="z")
                _ln(nc, sbufs, psums, smalls, eps_tile, rc, zf)

                # --- FFN ---
                h1 = acts.tile([128, TT], dt.bfloat16, tag="h1")
                for (c0, cn) in CHUNKS:
                    p = psums.tile([128, 512], dt.float32, tag="pB", bufs=2)
                    for d in range(DT):
                        nc.tensor.matmul(p[:, :cn], w["w1"][:, d, :],
                                         zf[:, d, c0:c0 + cn],
                                         start=(d == 0), stop=(d == DT - 1))
                    nc.scalar.activation(h1[:, c0:c0 + cn], p[:, :cn], AF.Relu,
                                         bias=bias["b1"][:], scale=1.0)
                y = acts.tile([128, DT, TT], dt.bfloat16, tag="q")
                for (c0, cn) in CHUNKS:
                    for o in range(DT):
                        p = psums.tile([128, 512], dt.float32, tag="pA", bufs=2)
                        nc.tensor.matmul(p[:, :cn], w["w2"][:, 128 * o:128 * o + 128],
                                         h1[:, c0:c0 + cn], start=True, stop=False)
                        nc.tensor.matmul(p[:, :cn], smalls["ident"][:],
                                         rc[:, o, c0:c0 + cn], start=False, stop=True)
                        nc.scalar.activation(y[:, o, c0:c0 + cn], p[:, :cn], AF.Identity,
                                             bias=bias["b2"][:, o:o + 1], scale=1.0)

                # --- ln_out -> next cat (with affine go/bo2 fused) ---
                cat_next = acts.tile([128, DT, TT], dt.bfloat16, tag=f"cat{(l + 1) % 2}")
                _ln(nc, sbufs, psums, smalls, eps_tile, y, cat_next,
                    affine=(bias["go"], bias["bo2"]))
                cat = cat_next

            # --- mean-pool utterance tokens (pos 1..4 of each 5-block) ---
            out_sb = sbufs.tile([128, DT], dt.float32, tag="outsb")
            for d in range(DT):
                view = cat[:, d, :].rearrange("p (s j) -> p s j", j=5)[:, :, 1:5]
                nc.vector.tensor_reduce(out_sb[:, d:d + 1], view,
                                        axis=mybir.AxisListType.XY, op=ALU.add)
            nc.vector.tensor_scalar_mul(out_sb[:], out_sb[:], 1.0 / U)
            nc.sync.dma_start(out_dram, out_sb[:])
    nc.compile()
    return nc


def _build():
    nc = bacc.Bacc("TRN2", target_bir_lowering=False, debug=False, num_devices=NCORES)
    return _trace(nc)


def kernel(**inputs):
    global _COMPILED
    ins = {k: np.asarray(v) for k, v in inputs.items()}
    shared = _host_prep(ins)
    idx = _tok_index()
    x = ins["x"].astype(np.float32)          # [B, T, D]
    xp = x[:, idx, :]                        # [B, TT, D]
    xT = np.ascontiguousarray(xp.transpose(0, 2, 1)).astype(bf16)  # [B, D, TT]
    if _COMPILED is None:
        _COMPILED = _build()
    nc = _COMPILED
    in_maps = []
    for b in range(NCORES):
        m = dict(shared)
        m["xT"] = xT[b]
        in_maps.append(m)
    res = bass_utils.run_bass_kernel_spmd(nc, in_maps, core_ids=list(range(NCORES)))
    outs = []
    for b in range(NCORES):
        o = res.results[b]["out"]            # [128, DT]
        outs.append(o.T.reshape(D))          # d = dtile*128 + p
    return np.stack(outs).astype(np.float32)


# revision 23
# speedup vs baseline: 1.7411x; 1.1784x over previous
"""Trainium2 Bass kernel for nn_BasicNet4 (Emformer encoder, sparse attention).

Strategy:
  - Data-parallel over batch B=8 across 8 NeuronCores (weights replicated).
  - Tokens reordered host-side into segment-interleaved order:
    seg i -> [rc_i, u_{4i}, u_{4i+1}, u_{4i+2}, u_{4i+3}]  (5 tokens x 256 segs = 1280)
    so attention is block-diagonal with 5x5 blocks.
  - Attention computed in 125-token windows (25 whole segments) so every
    window has the SAME block-diagonal mask (rank-26 factorization folded
    into the scores matmul) and there are no cross-window halos/edges.
  - Activations kept transposed in SBUF: [d on partitions (4 tiles of 128), tokens on free].
  - LayerNorm gains/biases folded into weights host-side; LN stats computed with
    ones-matmul partition reductions on the PE (broadcast form), variance finished
    in-PSUM via a K=1 matmul of -mu^2, and rstd = exp(-0.5*ln(var+eps)) on the
    scalar engine (no reciprocal, no sqrt table set).
  - V bias folded through Wo into bo host-side (softmax rows sum to 1).
  - All stages chunked (512-col chunks / per-window) with small rotating
    PSUM tags so the tile scheduler can pipeline engines 2-deep.
  - bf16 matmul operands / residual stream, fp32 PSUM accumulation.
"""

import sys

sys.path.insert(0, "/opt/trn_rl_repo")

import numpy as np
import ml_dtypes

import concourse.bass as bass
import concourse.mybir as mybir
import concourse.tile as tile
from concourse import bass_utils, bacc

bf16 = ml_dtypes.bfloat16
dt = mybir.dt
AF = mybir.ActivationFunctionType
ALU = mybir.AluOpType

# Model config (hardcoded from the problem spec)
D, H, FFN, L = 512, 4, 128, 4
SEG, RC = 4, 1
B, T = 8, 1025
U = T - RC            # 1024
NSEG = U // SEG       # 256
TT = NSEG * (SEG + RC)  # 1280 interleaved tokens
DT = D // 128         # 4 d tiles
DH = D // H           # 128 (= one partition tile per head)
NCORES = 8
CHUNKS = [(0, 512), (512, 512), (1024, 256)]  # free-dim chunks <= 512 (one PSUM bank)

W = 125               # attention window = 25 whole segments
NWF = TT // W         # 10 full windows
WL = TT - NWF * W     # 30-token last window (6 segments)
WINDOWS = [(w * W, W) for w in range(NWF)] + [(NWF * W, WL)]
NW = len(WINDOWS)     # 11

CBF = np.float32(bf16(np.float32(1e9)))  # mask constant, exactly representable in bf16

_COMPILED = None


def _tok_index():
    # interleaved token t -> original frame index in x[:, :T]
    t = np.arange(TT)
    seg = t // 5
    pos = t % 5
    off = np.array([4, 0, 1, 2, 3])[pos]
    return 4 * seg + off  # in [0, 1024]


def _mask_consts():
    """Uniform per-window mask factorization.
    Lm [26, W] (lhsT), Rm [26, W] (rhs): sum_r Lm[r,k]*Rm[r,j] =
    -C + C*[seg(k)==seg(j)] for window-local tokens. Every 125-token window
    holds exactly 25 segments, so one constant serves all full windows; the
    30-token last window uses the top-left slice with 6+1 rows.
    """
    nseg = W // 5
    seg = np.arange(W) // 5
    Lm = np.zeros((1 + nseg, W), np.float32)
    Rm = np.zeros((1 + nseg, W), np.float32)
    Lm[0, :] = 1.0
    Rm[0, :] = -CBF
    for i in range(nseg):
        Lm[1 + i] = (seg == i)
        Rm[1 + i] = CBF * (seg == i)
    return Lm.astype(bf16), Rm.astype(bf16)


def _host_prep(ins):
    """Fold LN affines into weights, transpose, cast. Returns shared input map."""
    f32 = np.float32
    m = {}
    scale = np.float32(DH) ** -0.5
    for l in range(L):
        g_i, b_i = f32(ins["ln_in_g"][l]), f32(ins["ln_in_b"][l])
        g_f, b_f = f32(ins["ff_ln_g"][l]), f32(ins["ff_ln_b"][l])
        Wq = f32(ins["Wq"][l]);  bq = f32(ins["bq"][l])
        Wk = f32(ins["Wkv"][l][:D]);  bk = f32(ins["bkv"][l][:D])
        Wv = f32(ins["Wkv"][l][D:]);  bv = f32(ins["bkv"][l][D:])
        Wo = f32(ins["Wo"][l]);  bo = f32(ins["bo"][l])
        W1 = f32(ins["W1"][l]);  b1 = f32(ins["b1"][l])
        W2 = f32(ins["W2"][l]);  b2 = f32(ins["b2"][l])
        Wq_ = scale * (Wq * g_i[None, :]); bq_ = scale * (bq + Wq @ b_i)
        Wk_ = Wk * g_i[None, :];           bk_ = bk + Wk @ b_i
        Wv_ = Wv * g_i[None, :];           bv_ = bv + Wv @ b_i
        # softmax rows sum to 1, so attention(v + bv) = attention(v) + bv;
        # fold bv through Wo into bo and drop the on-device v-bias entirely.
        bo = bo + Wo @ bv_
        W1_ = W1 * g_f[None, :];           b1_ = b1 + W1 @ b_f
        m[f"wq{l}"] = Wq_.T.copy().astype(bf16)   # [din, dout]
        m[f"wk{l}"] = Wk_.T.copy().astype(bf16)
        m[f"wv{l}"] = Wv_.T.copy().astype(bf16)
        m[f"wo{l}"] = Wo.T.copy().astype(bf16)
        m[f"w1{l}"] = W1_.T.copy().astype(bf16)   # [512, 128]
        m[f"w2{l}"] = W2.T.copy().astype(bf16)    # [128, 512]
        m[f"bq{l}"] = bq_.reshape(DT, 128).T.copy()       # [128, DT] f32 per-partition
        m[f"bk{l}"] = bk_.reshape(DT, 128).T.copy()
        m[f"bo{l}"] = bo.reshape(DT, 128).T.copy()
        m[f"b1{l}"] = b1_.reshape(1, 128).T.copy()        # [128, 1]
        m[f"b2{l}"] = b2.reshape(DT, 128).T.copy()
        m[f"go{l}"] = f32(ins["ln_out_g"][l]).reshape(DT, 128).T.copy()
        m[f"bo2{l}"] = f32(ins["ln_out_b"][l]).reshape(DT, 128).T.copy()
    Lm, Rm = _mask_consts()
    m["lmask"] = np.ascontiguousarray(Lm)                  # [26, W]
    m["rmask4"] = np.ascontiguousarray(
        np.tile(Rm[:, None, :], (1, H, 1)).reshape(Rm.shape[0], H * W)
    )                                                      # [26, H*W] (per-head blocks)
    m["ones_c"] = np.full((128, 128), 1.0 / D, bf16)       # stats lhsT (bcast reduce)
    m["allones"] = np.ones((128, 128), bf16)               # denominator lhsT
    m["onesneg"] = np.full((1, 128), -1.0, bf16)           # K=1 -mu^2 accumulation lhsT
    m["ident"] = np.eye(128, dtype=bf16)                   # residual adds
    return m


def _dram_inputs(nc):
    a = {}
    def inp(name, shape, dtype):
        a[name] = nc.dram_tensor(name, list(shape), dtype, kind="ExternalInput").ap()
    inp("xT", (D, TT), dt.bfloat16)
    for l in range(L):
        inp(f"wq{l}", (D, D), dt.bfloat16); inp(f"wk{l}", (D, D), dt.bfloat16)
        inp(f"wv{l}", (D, D), dt.bfloat16); inp(f"wo{l}", (D, D), dt.bfloat16)
        inp(f"w1{l}", (D, FFN), dt.bfloat16); inp(f"w2{l}", (FFN, D), dt.bfloat16)
        inp(f"bq{l}", (128, DT), dt.float32); inp(f"bk{l}", (128, DT), dt.float32)
        inp(f"bo{l}", (128, DT), dt.float32)
        inp(f"b1{l}", (128, 1), dt.float32); inp(f"b2{l}", (128, DT), dt.float32)
        inp(f"go{l}", (128, DT), dt.float32); inp(f"bo2{l}", (128, DT), dt.float32)
    inp("lmask", (26, W), dt.bfloat16); inp("rmask4", (26, H * W), dt.bfloat16)
    inp("ones_c", (128, 128), dt.bfloat16); inp("allones", (128, 128), dt.bfloat16)
    inp("onesneg", (1, 128), dt.bfloat16); inp("ident", (128, 128), dt.bfloat16)
    out = nc.dram_tensor("out", [128, DT], dt.float32, kind="ExternalOutput").ap()
    return a, out


def _ln(nc, sbufs, psums, smalls, eps_tile, src, dst, affine=None):
    """dst = (src - mean) * rstd [* go + bo2], chunked over tokens.
    src/dst: [128, DT, TT] bf16 sbuf. Stats per chunk: mu and E[x^2] via
    ones-matmuls; -mu^2 accumulated into the E[x^2] psum by a K=1 matmul so
    the psum holds var; rstd = exp(-0.5*ln(var+eps)) on the scalar engine."""
    ones_c, onesneg = smalls["ones_c"], smalls["onesneg"]
    for ci, (c0, cn) in enumerate(CHUNKS):
        sqs = []
        for d in range(DT):
            sqt = sbufs.tile([128, 512], dt.bfloat16, tag=f"sq{d % 2}", bufs=2)
            eng = (nc.vector, nc.gpsimd, nc.vector, nc.gpsimd)[d]
            eng.tensor_tensor(sqt[:, :cn], src[:, d, c0:c0 + cn],
                              src[:, d, c0:c0 + cn], ALU.mult)
            sqs.append(sqt)
        p_mu = psums.tile([128, 512], dt.float32, tag="stA", bufs=2)
        p_e2 = psums.tile([128, 512], dt.float32, tag="stB", bufs=2)
        for d in range(DT):
            nc.tensor.matmul(p_mu[:, :cn], ones_c[:], src[:, d, c0:c0 + cn],
                             start=(d == 0), stop=(d == DT - 1))
        for d in range(DT):
            nc.tensor.matmul(p_e2[:, :cn], ones_c[:], sqs[d][:, :cn],
                             start=(d == 0), stop=False)
        mu_b = sbufs.tile([128, 512], dt.bfloat16, tag="mu", bufs=2)
        nc.vector.tensor_copy(mu_b[:, :cn], p_mu[:, :cn])
        musq = sbufs.tile([1, 512], dt.bfloat16, tag="musq", bufs=2)
        nc.vector.tensor_tensor(musq[0:1, :cn], mu_b[0:1, :cn], mu_b[0:1, :cn],
                                ALU.mult)
        nc.tensor.matmul(p_e2[:, :cn], onesneg[:], musq[0:1, :cn],
                         start=False, stop=True)   # p_e2 now holds var
        sd = sbufs.tile([128, 512], dt.float32, tag="lnv", bufs=2)
        nc.scalar.activation(sd[:, :cn], p_e2[:, :cn], AF.Sqrt,
                             bias=eps_tile[:], scale=1.0)
        Af = sbufs.tile([128, 512], dt.float32, tag="Af", bufs=2)
        nc.vector.reciprocal_approx_fast(Af[:, :cn], sd[:, :cn])
        A = sbufs.tile([128, 512], dt.bfloat16, tag="A", bufs=2)
        nc.vector.tensor_copy(A[:, :cn], Af[:, :cn])
        for d in range(DT):
            z1 = sbufs.tile([128, 512], dt.bfloat16, tag=f"z1{d % 2}", bufs=2)
            e1 = ((nc.vector, nc.gpsimd, nc.gpsimd, nc.vector)
                  if affine is not None else
                  (nc.vector, nc.gpsimd, nc.vector, nc.vector))[d]
            e1.tensor_tensor(z1[:, :cn], src[:, d, c0:c0 + cn], mu_b[:, :cn],
                             ALU.subtract)
            if affine is None:
                e2 = (nc.vector, nc.vector, nc.gpsimd, nc.vector)[d]
                e2.tensor_tensor(dst[:, d, c0:c0 + cn], z1[:, :cn], A[:, :cn],
                                 ALU.mult)
            else:
                go, bo2 = affine
                nc.vector.scalar_tensor_tensor(dst[:, d, c0:c0 + cn], z1[:, :cn],
                                               go[:, d:d + 1], A[:, :cn],
                                               ALU.mult, ALU.mult)
                nc.vector.tensor_scalar_add(dst[:, d, c0:c0 + cn],
                                            dst[:, d, c0:c0 + cn],
                                            bo2[:, d:d + 1])


def _trace(nc):
    a, out_dram = _dram_inputs(nc)
    with tile.TileContext(nc) as tc:
        import contextlib
        ctx = contextlib.ExitStack()
        with ctx:
            consts = ctx.enter_context(tc.tile_pool(name="consts", bufs=1))
            wpool = ctx.enter_context(tc.tile_pool(name="w", bufs=2))
            acts = ctx.enter_context(tc.tile_pool(name="acts", bufs=1))
            sbufs = ctx.enter_context(tc.tile_pool(name="sbufs", bufs=2))
            psums = ctx.enter_context(tc.tile_pool(name="psums", bufs=2, space="PSUM"))

            # constants
            smalls = {}
            for name, shape, dd in [
                ("lmask", [26, W], dt.bfloat16), ("rmask4", [26, H, W], dt.bfloat16),
                ("ones_c", [128, 128], dt.bfloat16), ("allones", [128, 128], dt.bfloat16),
                ("onesneg", [1, 128], dt.bfloat16), ("ident", [128, 128], dt.bfloat16),
            ]:
                t = consts.tile(shape, dd, tag=name)
                src = a[name]
                if name == "rmask4":
                    src = src.rearrange("r (h w) -> r h w", h=H)
                nc.sync.dma_start(t[:], src)
                smalls[name] = t
            eps_tile = consts.tile([128, 1], dt.float32)
            nc.vector.memset(eps_tile[:], 1e-5)

            # initial residual stream (transposed, interleaved)
            cat = acts.tile([128, DT, TT], dt.bfloat16, tag="cat0")
            nc.sync.dma_start(cat[:], a["xT"].rearrange("(dtile p) t -> p dtile t", p=128))

            for l in range(L):
                # --- load layer weights ---
                w = {}
                for nm, shape in [("wq", [128, DT, D]), ("wk", [128, DT, D]),
                                  ("wv", [128, DT, D]), ("wo", [128, DT, D]),
                                  ("w1", [128, DT, FFN]), ("w2", [128, D])]:
                    t = wpool.tile(shape, dt.bfloat16, tag=nm)
                    src = a[f"{nm}{l}"]
                    if nm == "w2":
                        nc.sync.dma_start(t[:], src)
                    else:
                        nc.sync.dma_start(t[:], src.rearrange("(dtile p) o -> p dtile o", p=128))
                    w[nm] = t
                bias = {}
                for nm in ["bq", "bk", "bo", "b1", "b2", "go", "bo2"]:
                    t = wpool.tile([128, DT] if nm != "b1" else [128, 1], dt.float32, tag=nm)
                    nc.sync.dma_start(t[:], a[f"{nm}{l}"])
                    bias[nm] = t

                # --- ln_in -> z ---
                z = acts.tile([128, DT, TT], dt.bfloat16, tag="z")
                _ln(nc, sbufs, psums, smalls, eps_tile, cat, z)

                # --- Q, K projections (weights stationary -> transposed out) ---
                qk = {}
                for nm, bnm in [("wq", "bq"), ("wk", "bk")]:
                    dst = acts.tile([128, DT, TT], dt.bfloat16, tag="q" if nm == "wq" else "k")
                    for (c0, cn) in CHUNKS:
                        for o in range(DT):
                            p = psums.tile([128, 512], dt.float32, tag="pA", bufs=2)
                            for d in range(DT):
                                nc.tensor.matmul(
                                    p[:, :cn],
                                    w[nm][:, d, 128 * o:128 * o + 128],
                                    z[:, d, c0:c0 + cn],
                                    start=(d == 0), stop=(d == DT - 1))
                            if nm == "wq":
                                nc.scalar.activation(dst[:, o, c0:c0 + cn], p[:, :cn],
                                                     AF.Identity,
                                                     bias=bias[bnm][:, o:o + 1], scale=1.0)
                            else:
                                nc.vector.tensor_scalar_add(dst[:, o, c0:c0 + cn],
                                                            p[:, :cn],
                                                            bias[bnm][:, o:o + 1])
                    qk[nm] = dst
                q_t, k_t = qk["wq"], qk["wk"]

                # --- V projection (acts stationary -> natural out per window) ---
                v_nat = acts.tile([128, NW, D], dt.bfloat16, tag="v")
                for wi, (w0, wn) in enumerate(WINDOWS):
                    p = psums.tile([128, 512], dt.float32, tag="pB", bufs=2)
                    for d in range(DT):
                        nc.tensor.matmul(p[0:wn, :], z[:, d, w0:w0 + wn],
                                         w["wv"][:, d, :], start=(d == 0),
                                         stop=(d == DT - 1))
                    nc.vector.tensor_copy(v_nat[0:wn, wi], p[0:wn, :])

                # --- attention per 125-token window (all heads share psums) ---
                attn = acts.tile([128, DT, TT], dt.bfloat16, tag="attn")
                lmask, rmask4, allones = smalls["lmask"], smalls["rmask4"], smalls["allones"]
                nsg = 1 + W // 5
                nsgl = 1 + WL // 5
                for wi, (w0, wn) in enumerate(WINDOWS):
                    nr = nsg if wn == W else nsgl
                    sc = psums.tile([128, H, W], dt.float32, tag="stA", bufs=2)
                    nc.tensor.matmul(sc[0:wn, :, 0:wn], lmask[0:nr, 0:wn],
                                     rmask4[0:nr, :, 0:wn], start=True, stop=False)
                    for h in range(H):
                        nc.tensor.matmul(sc[0:wn, h, 0:wn], k_t[:, h, w0:w0 + wn],
                                         q_t[:, h, w0:w0 + wn],
                                         start=False, stop=(h == H - 1))
                    pa = sbufs.tile([128, H, W], dt.bfloat16, tag="pa", bufs=2)
                    nc.scalar.activation(pa[0:wn, :, 0:wn], sc[0:wn, :, 0:wn], AF.Exp)
                    den = psums.tile([128, H, W], dt.float32, tag="stB", bufs=2)
                    p_av = psums.tile([128, H, W], dt.float32, tag="pA", bufs=2)
                    nc.tensor.matmul(den[:, :, 0:wn], allones[0:wn, :],
                                     pa[0:wn, :, 0:wn], start=True, stop=True)
                    for h in range(H):
                        nc.tensor.matmul(p_av[:, h, 0:wn],
                                         v_nat[0:wn, wi, 128 * h:128 * h + 128],
                                         pa[0:wn, h, 0:wn], start=True, stop=True)
                    rec = sbufs.tile([128, H, W], dt.float32, tag="rec", bufs=2)
                    nc.vector.reciprocal_approx_fast(rec[:, :, 0:wn], den[:, :, 0:wn])
                    nc.vector.tensor_tensor(attn[:, :, w0:w0 + wn], p_av[:, :, 0:wn],
                                            rec[:, :, 0:wn], ALU.mult)

                # --- Wo projection + residual ---
                rc = acts.tile([128, DT, TT], dt.bfloat16, tag=f"cat{(l + 1) % 2}")
                for (c0, cn) in CHUNKS:
                    for o in range(DT):
                        p = psums.tile([128, 512], dt.float32, tag="pA", bufs=2)
                        for d in range(DT):
                            nc.tensor.matmul(p[:, :cn],
                                             w["wo"][:, d, 128 * o:128 * o + 128],
                                             attn[:, d, c0:c0 + cn],
                                             start=(d == 0), stop=False)
                        nc.tensor.matmul(p[:, :cn], smalls["ident"][:],
                                         cat[:, o, c0:c0 + cn], start=False, stop=True)
                        nc.scalar.activation(rc[:, o, c0:c0 + cn], p[:, :cn], AF.Identity,
                                             bias=bias["bo"][:, o:o + 1], scale=1.0)

                # --- ff_ln -> zf ---
                zf = acts.tile([128, DT, TT], dt.bfloat16, tag="z")
                _ln(nc, sbufs, psums, smalls, eps_tile, rc, zf)

                # --- FFN ---
                h1 = acts.tile([128, TT], dt.bfloat16, tag="h1")
                for (c0, cn) in CHUNKS:
                    p = psums.tile([128, 512], dt.float32, tag="pB", bufs=2)
                    for d in range(DT):
                        nc.tensor.matmul(p[:, :cn], w["w1"][:, d, :],
                                         zf[:, d, c0:c0 + cn],
                                         start=(d == 0), stop=(d == DT - 1))
                    nc.scalar.activation(h1[:, c0:c0 + cn], p[:, :cn], AF.Relu,
                                         bias=bias["b1"][:], scale=1.0)
                y = acts.tile([128, DT, TT], dt.bfloat16, tag="q")
                for (c0, cn) in CHUNKS:
                    for o in range(DT):
                        p = psums.tile([128, 512], dt.float32, tag="pA", bufs=2)
                        nc.tensor.matmul(p[:, :cn], w["w2"][:, 128 * o:128 * o + 128],
                                         h1[:, c0:c0 + cn], start=True, stop=False)
                        nc.tensor.matmul(p[:, :cn], smalls["ident"][:],
                                         rc[:, o, c0:c0 + cn], start=False, stop=True)
                        nc.scalar.activation(y[:, o, c0:c0 + cn], p[:, :cn], AF.Identity,
                                             bias=bias["b2"][:, o:o + 1], scale=1.0)

                # --- ln_out -> next cat (with affine go/bo2 fused) ---
                cat_next = acts.tile([128, DT, TT], dt.bfloat16, tag=f"cat{(l + 1) % 2}")
                _ln(nc, sbufs, psums, smalls, eps_tile, y, cat_next,
                    affine=(bias["go"], bias["bo2"]))
                cat = cat_next

            # --- mean-pool utterance tokens (pos 1..4 of each 5-block) ---
            out_sb = sbufs.tile([128, DT], dt.float32, tag="outsb")
            for d in range(DT):
                view = cat[:, d, :].rearrange("p (s j) -> p s j", j=5)[:, :, 1:5]
                nc.vector.tensor_reduce(out_sb[:, d:d + 1], view,
                                        axis=mybir.AxisListType.XY, op=ALU.add)
            nc.vector.tensor_scalar_mul(out_sb[:], out_sb[:], 1.0 / U)
            nc.sync.dma_start(out_dram, out_sb[:])
    nc.compile()
    return nc


def _build():
    nc = bacc.Bacc("TRN2", target_bir_lowering=False, debug=False, num_devices=NCORES)
    return _trace(nc)


def kernel(**inputs):
    global _COMPILED
    ins = {k: np.asarray(v) for k, v in inputs.items()}
    shared = _host_prep(ins)
    idx = _tok_index()
    x = ins["x"].astype(np.float32)          # [B, T, D]
    xp = x[:, idx, :]                        # [B, TT, D]
    xT = np.ascontiguousarray(xp.transpose(0, 2, 1)).astype(bf16)  # [B, D, TT]
    if _COMPILED is None:
        _COMPILED = _build()
    nc = _COMPILED
    in_maps = []
    for b in range(NCORES):
        m = dict(shared)
        m["xT"] = xT[b]
        in_maps.append(m)
    res = bass_utils.run_bass_kernel_spmd(nc, in_maps, core_ids=list(range(NCORES)))
    outs = []
    for b in range(NCORES):
        o = res.results[b]["out"]            # [128, DT]
        outs.append(o.T.reshape(D))          # d = dtile*128 + p
    return np.stack(outs).astype(np.float32)
